# revision 22
# baseline (speedup 1.0000x reference)
"""BrightnessLoss Trainium2 kernel (raw Bass, 8-core data parallel).

reference:
    V(x)   = max_c(clip(x, 0, 1))        over channel dim (RGB)
    result = mean(|V(pred) - V(target)|) over (N, H, W)

Identities used on device:
    clip(max(r,g,b),0,1) == max_c(clip(x,0,1))          (clip is monotone)
    W := relu(1 - relu(m)) == 1 - clip(m, 0, 1)
    |Vp - Vt| == |Wp - Wt|
    sum|Wp - Wt| == 2*sum max(Wp,Wt) - sum(Wp + Wt)

The stream is the roofline: ~25.2 MB of fp32 input per core, and the 16
SDMA engines cap at ~24 GB/s each with 4 KB packets (per-packet
overhead), a bit more with 8 KB packets. So the design goal is a gapless
two-ring DMA stream of the largest-possible contiguous runs, with compute
strictly faster than arrival:

  - DMA "groups" cover column ranges of each image; even groups ride the
    Sync HWDGE ring, odd groups the ACT HWDGE ring, each carrying pred
    then targ back-to-back (12.6 MB per ring). Image 0 leads with paired
    small groups (128/128/256/256/640/640 cols) so compute starts ~1 us
    into the stream with both rings' packet sizes matched; image 3
    trails with the mirror (640/640/256/256/128/128) so the closing
    dependency chain is short.
  - 6 group slots [P, 2, 3, w] (both sides side-by-side) keep each ring
    ~3 transfers deep, so the rings never run dry.
  - Compute "units" (<=1024 cols) subdivide groups. Per unit, both sides
    in one wide op:
        DVE TT   m = max(R2, G2)          [P, 2, w]
        DVE STT  u = max(max(m,0), B2)    [P, 2, w]
        ACT      W = Relu(1 - u) (bf16),  accum_out = sum(Wp)+sum(Wt)
        DVE STT  max(Wp, Wt) (bf16),      accum_out = sum
    DVE needs ~5.8 us per 1024-col unit vs ~7.3 us arrival, so it stays
    caught up and the tail after the last packet is just the last small
    unit's chain. Partials go out in two DMAs (bulk early, last units at
    the end). Host combines in float64.
"""

import numpy as np

N_CORES = 8
N_IMG = 4  # 32 / 8
C = 3
P = 128
F = 2048  # 512*512 / 128
N_PIX = 32 * 512 * 512
FC = 1024  # max compute-unit width
S_G = 6  # group slot depth: deep enough that ring issues wait only on
# ancient compute (STT of g-6), keeping both rings' queues full from t=0
# Small transfers are expensive two ways: sub-4KB DRAM runs drop the
# per-SDMA-engine rate, and each transfer carries a ~1.5us ring bubble.
# So the head/tail use the FEWEST small pieces that still let compute
# start early / finish promptly: one 256 + one 768 per edge, staggered
# across the two rings, uniform 1024s everywhere else.
HEAD_SPLIT = (256, 768, 1024)  # image 0 groups (sum = F)
# Tail tapers 1024/640/384 with the 640 ending the SP ring and the 384
# ending the ACT ring (~512-col-side stagger): the final arrivals
# telescope into DVE's remaining work instead of landing all at once.
TAIL_SPLIT = (1024, 640, 384)  # last image groups (sum = F)


def _plan():
    """groups: (img, col_off, width); units: (grp_idx, off_in_grp, width).
    One group = one DMA transfer per side = one compute unit (<= FC cols):
    4 KB DRAM runs already saturate the per-SDMA-engine rate, and unit-
    sized transfers keep the slot-WAR release chain fine-grained."""
    groups = []
    for img in range(N_IMG):
        if img == 0:
            widths = HEAD_SPLIT
        elif img == N_IMG - 1:
            widths = TAIL_SPLIT
        else:
            widths = (FC,) * (F // FC)
        o = 0
        for w in widths:
            groups.append((img, o, w))
            o += w
        assert o == F
    units = []
    for g, (_img, _off, w) in enumerate(groups):
        o = 0
        while o < w:
            uw = min(FC, w - o)
            units.append((g, o, uw))
            o += uw
    return groups, units


def _build_program():
    from contextlib import ExitStack

    import concourse.bass as bass
    import concourse.mybir as mybir

    fp32 = mybir.dt.float32
    bf16 = mybir.dt.bfloat16
    Alu = mybir.AluOpType
    Act = mybir.ActivationFunctionType

    groups, units = _plan()
    n_groups = len(groups)
    n_units = len(units)
    last_unit_of = {}
    for u, (g, _o, _w) in enumerate(units):
        last_unit_of[g] = u
    slot_w = [
        max(groups[g][2] for g in range(s, n_groups, S_G)) for s in range(S_G)
    ]

    # detect_race_conditions=False: the raw-mode CoreSim race detector can't
    # see same-engine program-order (DVE m1 -> STT RAW); hardware engines
    # execute in order.
    # The construction-time all_engine_barrier orders the const-tile memsets
    # against engines that read them; this kernel uses only instruction
    # immediates, so skip it and let the engines reach first work sooner.
    _orig_barrier = bass.Bass.all_engine_barrier
    bass.Bass.all_engine_barrier = lambda *a, **k: None
    try:
        nc = bass.Bass(
            "TRN2",
            target_bir_lowering=False,
            debug=False,
            detect_race_conditions=False,
        )
    finally:
        bass.Bass.all_engine_barrier = _orig_barrier
    pred = nc.dram_tensor("pred", [N_IMG, C, P, F], fp32, kind="ExternalInput").ap()
    targ = nc.dram_tensor("target", [N_IMG, C, P, F], fp32, kind="ExternalInput").ap()
    out = nc.dram_tensor(
        "partials", [P, 2 * n_units], fp32, kind="ExternalOutput"
    ).ap()

    with ExitStack() as ctx:
        sb = lambda name, shape, dt=fp32: ctx.enter_context(
            nc.sbuf_tensor(name, shape, dt)
        )
        sem = lambda name: ctx.enter_context(nc.semaphore(name))

        # one slot holds BOTH sides of a group: [P, side, chan, slot_w]
        inb = [sb(f"in{s}", [P, 2, C, slot_w[s]]) for s in range(S_G)]
        ub = [sb(f"u{s}", [P, 2 * FC]) for s in range(2)]
        wb = [sb(f"w{s}", [P, 2 * FC], bf16) for s in range(2)]
        m1 = sb("m1", [P, 2 * FC])
        scr = sb("stt_scratch", [P, FC], bf16)
        acc = sb("acc", [P, 2 * n_units])

        inp_sem = [sem("inp0"), sem("inp1")]  # pred side, by ring parity
        int_sem = [sem("int0"), sem("int1")]  # targ side, by ring parity
        u_sem = sem("u")  # +1 per unit after DVE STT (inb consumed)
        act_sem = sem("act")  # +1 per unit after ACT (ub consumed, wb+acc ready)
        gp_sem = sem("gp")  # +1 per unit after DVE accum (wb consumed)
        out_sem = sem("outd")

        def dma_in(eng, side_idx, g):
            img, off, w = groups[g]
            side = (pred, targ)[side_idx]
            s_sem = (inp_sem, int_sem)[side_idx]
            src = side[img, :, :, off : off + w].rearrange("c p f -> p c f")
            eng.dma_start(
                out=inb[g % S_G][:, side_idx, :, :w],
                in_=src,
            ).then_inc(s_sem[g % 2], 16)

        block = ctx.enter_context(nc.Block(no_gpsimd_drain=True))

        @block.sync
        def _(sync):
            # even units ride the SP ring (pred+targ back-to-back); odd units
            # are issued from the ACT stream (second HWDGE ring). The two
            # rings stay one unit apart, which keeps their DRAM read streams
            # decorrelated — issuing each side on its own ring measurably
            # tanks the aggregate rate.
            for g in range(0, n_groups, 2):
                if g >= S_G:
                    # WAR inb[g%S_G]: unit g-S_G's STT was its last reader
                    sync.wait_ge(u_sem, g - S_G + 1)
                dma_in(sync, 0, g)
                dma_in(sync, 1, g)
            if n_units > 2:
                # bulk of partials early; only the last 2 units' cols remain.
                # gp_sem >= k implies act_sem >= k (accum u waits ACT u), so
                # both engines' acc columns for units < k are final.
                sync.wait_ge(gp_sem, n_units - 2)
                sync.dma_start(
                    out=out[:, : 2 * (n_units - 2)],
                    in_=acc[:, : 2 * (n_units - 2)],
                ).then_inc(out_sem, 16)
            sync.wait_ge(gp_sem, n_units)
            # No out_sem wait after the final write: the block-exit drain
            # fences the HWDGE ring before NEFF completion.
            sync.dma_start(
                out=out[:, 2 * max(0, n_units - 2) :],
                in_=acc[:, 2 * max(0, n_units - 2) :],
            ).then_inc(out_sem, 16)

        @block.vector
        def _(vector):
            def accum(u):
                # max(Wp, Wt) elementwise (bf16), accum_out = row sum
                w = units[u][2]
                vector.wait_ge(act_sem, u + 1)
                vector.scalar_tensor_tensor(
                    scr[:, :w],
                    wb[u % 2][:, :w],
                    0.0,
                    wb[u % 2][:, w : 2 * w],
                    op0=Alu.bypass,
                    op1=Alu.max,
                    accum_out=acc[:, 2 * u : 2 * u + 1],
                ).then_inc(gp_sem, 1)

            # Units near the stream's edges run per-SIDE (pred ops while
            # targ is still in flight — pred lands half a group earlier), so
            # the residual work after the LAST byte arrives is halved. Mid-
            # stream units keep the fused wide ops (fewer op overheads).
            unfused = {0, n_units - 3, n_units - 2, n_units - 1}

            def side_ops(u, s):
                g, o, w = units[u]
                t = inb[g % S_G]
                vector.tensor_max(
                    m1[:, s * w : (s + 1) * w],
                    t[:, s, 0, o : o + w],
                    t[:, s, 1, o : o + w],
                )
                st = vector.scalar_tensor_tensor(
                    ub[u % 2][:, s * w : (s + 1) * w],
                    m1[:, s * w : (s + 1) * w],
                    0.0,
                    t[:, s, 2, o : o + w],
                    op0=Alu.max,
                    op1=Alu.max,
                )
                return st

            for u in range(n_units):
                g, o, w = units[u]
                t = inb[g % S_G]
                if u in unfused:
                    vector.wait_ge(inp_sem[g % 2], 16 * (g // 2 + 1))
                    if u >= 2:
                        # WAR on ub[u%2]: ACT's W of unit u-2 (its reader)
                        vector.wait_ge(act_sem, u - 1)
                    side_ops(u, 0)
                    vector.wait_ge(int_sem[g % 2], 16 * (g // 2 + 1))
                    side_ops(u, 1).then_inc(u_sem, 1)
                    if u > 0:
                        # accum AFTER both sides: its act_sem wait must not
                        # gate the targ-side ops (that serializes the tail)
                        accum(u - 1)
                else:
                    vector.wait_ge(inp_sem[g % 2], 16 * (g // 2 + 1))
                    vector.wait_ge(int_sem[g % 2], 16 * (g // 2 + 1))
                    mv = m1[:, : 2 * w].rearrange("p (s w) -> p s w", s=2)
                    uv = ub[u % 2][:, : 2 * w].rearrange("p (s w) -> p s w", s=2)
                    vector.tensor_max(
                        mv, t[:, :, 0, o : o + w], t[:, :, 1, o : o + w]
                    )
                    if u >= 2:
                        vector.wait_ge(act_sem, u - 1)
                    vector.scalar_tensor_tensor(
                        uv,
                        mv,
                        0.0,
                        t[:, :, 2, o : o + w],
                        op0=Alu.max,
                        op1=Alu.max,
                    ).then_inc(u_sem, 1)
                    accum(u - 1)
            accum(n_units - 1)

        @block.scalar
        def _(scalar):
            # odd units' input DMAs ride the ACT HWDGE ring. Units 1 and 3 go
            # up front (fresh slots, no WAR); unit n+S_G is placed right
            # after ACT(n), whose u_sem wait (>= n+1) covers the WAR for slot
            # (n+S_G) % S_G (last STT reader was unit n).
            for g in range(1, min(S_G, n_groups), 2):
                dma_in(scalar, 0, g)
                dma_in(scalar, 1, g)
            for n in range(n_units):
                w = units[n][2]
                scalar.wait_ge(u_sem, n + 1)
                if n >= 2:
                    # WAR on wb[n%2]: accum of unit n-2 (its reader)
                    scalar.wait_ge(gp_sem, n - 1)
                scalar.activation(
                    wb[n % 2][:, : 2 * w],
                    ub[n % 2][:, : 2 * w],
                    Act.Relu,
                    bias=1.0,
                    scale=-1.0,
                    accum_out=acc[:, 2 * n + 1 : 2 * n + 2],
                ).then_inc(act_sem, 1)
                if n + S_G < n_groups and (n + S_G) % 2 == 1:
                    dma_in(scalar, 0, n + S_G)
                    dma_in(scalar, 1, n + S_G)

        # Skip the Block-exit all-engine barrier (~4.3us): every cross-engine
        # dependency is semaphore-gated and the per-engine exit drains
        # (no_gpsimd_drain path) still fence the DMA rings, so engines may
        # halt independently — NEFF completion waits for all engines anyway.
        nc.all_engine_barrier = lambda *a, **k: None

    del nc.all_engine_barrier  # restore class method
    return nc


_program = None


def _get_program():
    global _program
    if _program is None:
        _program = _build_program()
    return _program


def _finish(partials_list):
    """partials_list: per-core [P, 2*n_units] f32 with cols per unit:
    [sum max(Wp,Wt), sum Wp + sum Wt].
    sum|Vp-Vt| = 2*sum(max) - (sum Wp + sum Wt)."""
    total = np.float64(0.0)
    for p in partials_list:
        p = p.astype(np.float64)
        total += 2.0 * p[:, 0::2].sum() - p[:, 1::2].sum()
    return np.array(total / N_PIX, dtype=np.float32)


def kernel(pred: np.ndarray, target: np.ndarray) -> np.ndarray:
    from concourse.bass_utils import run_bass_kernel_spmd

    nc = _get_program()
    pred = np.ascontiguousarray(pred, dtype=np.float32).reshape(
        N_CORES, N_IMG, C, P, F
    )
    target = np.ascontiguousarray(target, dtype=np.float32).reshape(
        N_CORES, N_IMG, C, P, F
    )
    in_maps = [{"pred": pred[i], "target": target[i]} for i in range(N_CORES)]
    res = run_bass_kernel_spmd(nc, in_maps, list(range(N_CORES)))
    return _finish([r["partials"] for r in res.results])


# revision 23
# speedup vs baseline: 1.0036x; 1.0036x over previous
"""BrightnessLoss Trainium2 kernel (raw Bass, 8-core data parallel).

reference:
    V(x)   = max_c(clip(x, 0, 1))        over channel dim (RGB)
    result = mean(|V(pred) - V(target)|) over (N, H, W)

Identities used on device:
    clip(max(r,g,b),0,1) == max_c(clip(x,0,1))          (clip is monotone)
    W := relu(1 - relu(m)) == 1 - clip(m, 0, 1)
    |Vp - Vt| == |Wp - Wt|
    sum|Wp - Wt| == 2*sum max(Wp,Wt) - sum(Wp + Wt)

The stream is the roofline: ~25.2 MB of fp32 input per core, and the 16
SDMA engines cap at ~24 GB/s each with 4 KB packets (per-packet
overhead), a bit more with 8 KB packets. So the design goal is a gapless
two-ring DMA stream of the largest-possible contiguous runs, with compute
strictly faster than arrival:

  - DMA "groups" cover column ranges of each image; even groups ride the
    Sync HWDGE ring, odd groups the ACT HWDGE ring, each carrying pred
    then targ back-to-back (12.6 MB per ring). Image 0 leads with paired
    small groups (128/128/256/256/640/640 cols) so compute starts ~1 us
    into the stream with both rings' packet sizes matched; image 3
    trails with the mirror (640/640/256/256/128/128) so the closing
    dependency chain is short.
  - 6 group slots [P, 2, 3, w] (both sides side-by-side) keep each ring
    ~3 transfers deep, so the rings never run dry.
  - Compute "units" (<=1024 cols) subdivide groups. Per unit, both sides
    in one wide op:
        DVE TT   m = max(R2, G2)          [P, 2, w]
        DVE STT  u = max(max(m,0), B2)    [P, 2, w]
        ACT      W = Relu(1 - u) (bf16),  accum_out = sum(Wp)+sum(Wt)
        DVE STT  max(Wp, Wt) (bf16),      accum_out = sum
    DVE needs ~5.8 us per 1024-col unit vs ~7.3 us arrival, so it stays
    caught up and the tail after the last packet is just the last small
    unit's chain. Partials go out in two DMAs (bulk early, last units at
    the end). Host combines in float64.
"""

import numpy as np

N_CORES = 8
N_IMG = 4  # 32 / 8
C = 3
P = 128
F = 2048  # 512*512 / 128
N_PIX = 32 * 512 * 512
FC = 1024  # max compute-unit width
S_G = 6  # group slot depth: deep enough that ring issues wait only on
# ancient compute (STT of g-6), keeping both rings' queues full from t=0
# Small transfers are expensive two ways: sub-4KB DRAM runs drop the
# per-SDMA-engine rate, and each transfer carries a ~1.5us ring bubble.
# So the head/tail use the FEWEST small pieces that still let compute
# start early / finish promptly: one 256 + one 768 per edge, staggered
# across the two rings, uniform 1024s everywhere else.
HEAD_SPLIT = (256, 768, 1024)  # image 0 groups (sum = F)
TAIL_SPLIT = (1024, 768, 256)  # last image groups (sum = F)


def _plan():
    """groups: (img, col_off, width); units: (grp_idx, off_in_grp, width).
    One group = one DMA transfer per side = one compute unit (<= FC cols):
    4 KB DRAM runs already saturate the per-SDMA-engine rate, and unit-
    sized transfers keep the slot-WAR release chain fine-grained."""
    groups = []
    for img in range(N_IMG):
        if img == 0:
            widths = HEAD_SPLIT
        elif img == N_IMG - 1:
            widths = TAIL_SPLIT
        else:
            widths = (FC,) * (F // FC)
        o = 0
        for w in widths:
            groups.append((img, o, w))
            o += w
        assert o == F
    units = []
    for g, (_img, _off, w) in enumerate(groups):
        o = 0
        while o < w:
            uw = min(FC, w - o)
            units.append((g, o, uw))
            o += uw
    return groups, units


def _build_program():
    from contextlib import ExitStack

    import concourse.bass as bass
    import concourse.mybir as mybir

    fp32 = mybir.dt.float32
    bf16 = mybir.dt.bfloat16
    Alu = mybir.AluOpType
    Act = mybir.ActivationFunctionType

    groups, units = _plan()
    n_groups = len(groups)
    n_units = len(units)
    last_unit_of = {}
    for u, (g, _o, _w) in enumerate(units):
        last_unit_of[g] = u
    slot_w = [
        max(groups[g][2] for g in range(s, n_groups, S_G)) for s in range(S_G)
    ]

    # detect_race_conditions=False: the raw-mode CoreSim race detector can't
    # see same-engine program-order (DVE m1 -> STT RAW); hardware engines
    # execute in order.
    # The construction-time all_engine_barrier orders the const-tile memsets
    # against engines that read them; this kernel uses only instruction
    # immediates, so skip it and let the engines reach first work sooner.
    _orig_barrier = bass.Bass.all_engine_barrier
    bass.Bass.all_engine_barrier = lambda *a, **k: None
    try:
        nc = bass.Bass(
            "TRN2",
            target_bir_lowering=False,
            debug=False,
            detect_race_conditions=False,
        )
    finally:
        bass.Bass.all_engine_barrier = _orig_barrier
    pred = nc.dram_tensor("pred", [N_IMG, C, P, F], fp32, kind="ExternalInput").ap()
    targ = nc.dram_tensor("target", [N_IMG, C, P, F], fp32, kind="ExternalInput").ap()
    out = nc.dram_tensor(
        "partials", [P, 2 * n_units], fp32, kind="ExternalOutput"
    ).ap()

    with ExitStack() as ctx:
        sb = lambda name, shape, dt=fp32: ctx.enter_context(
            nc.sbuf_tensor(name, shape, dt)
        )
        sem = lambda name: ctx.enter_context(nc.semaphore(name))

        # one slot holds BOTH sides of a group: [P, side, chan, slot_w]
        inb = [sb(f"in{s}", [P, 2, C, slot_w[s]]) for s in range(S_G)]
        ub = [sb(f"u{s}", [P, 2 * FC]) for s in range(2)]
        wb = [sb(f"w{s}", [P, 2 * FC], bf16) for s in range(2)]
        m1 = sb("m1", [P, 2 * FC])
        scr = sb("stt_scratch", [P, FC], bf16)
        acc = sb("acc", [P, 2 * n_units])

        inp_sem = [sem("inp0"), sem("inp1")]  # pred side, by ring parity
        int_sem = [sem("int0"), sem("int1")]  # targ side, by ring parity
        u_sem = sem("u")  # +1 per unit after DVE STT (inb consumed)
        act_sem = sem("act")  # +1 per unit after ACT (ub consumed, wb+acc ready)
        gp_sem = sem("gp")  # +1 per unit after DVE accum (wb consumed)
        out_sem = sem("outd")

        def dma_in(eng, side_idx, g):
            img, off, w = groups[g]
            side = (pred, targ)[side_idx]
            s_sem = (inp_sem, int_sem)[side_idx]
            src = side[img, :, :, off : off + w].rearrange("c p f -> p c f")
            eng.dma_start(
                out=inb[g % S_G][:, side_idx, :, :w],
                in_=src,
            ).then_inc(s_sem[g % 2], 16)

        block = ctx.enter_context(nc.Block(no_gpsimd_drain=True))

        @block.sync
        def _(sync):
            # even units ride the SP ring (pred+targ back-to-back); odd units
            # are issued from the ACT stream (second HWDGE ring). The two
            # rings stay one unit apart, which keeps their DRAM read streams
            # decorrelated — issuing each side on its own ring measurably
            # tanks the aggregate rate.
            for g in range(0, n_groups, 2):
                if g >= S_G:
                    # WAR inb[g%S_G]: unit g-S_G's STT was its last reader
                    sync.wait_ge(u_sem, g - S_G + 1)
                dma_in(sync, 0, g)
                dma_in(sync, 1, g)
            if n_units > 2:
                # bulk of partials early; only the last 2 units' cols remain.
                # gp_sem >= k implies act_sem >= k (accum u waits ACT u), so
                # both engines' acc columns for units < k are final.
                sync.wait_ge(gp_sem, n_units - 2)
                sync.dma_start(
                    out=out[:, : 2 * (n_units - 2)],
                    in_=acc[:, : 2 * (n_units - 2)],
                ).then_inc(out_sem, 16)
            sync.wait_ge(gp_sem, n_units)
            # No out_sem wait after the final write: the block-exit drain
            # fences the HWDGE ring before NEFF completion.
            sync.dma_start(
                out=out[:, 2 * max(0, n_units - 2) :],
                in_=acc[:, 2 * max(0, n_units - 2) :],
            ).then_inc(out_sem, 16)

        @block.vector
        def _(vector):
            def accum(u):
                # max(Wp, Wt) elementwise (bf16), accum_out = row sum
                w = units[u][2]
                vector.wait_ge(act_sem, u + 1)
                vector.scalar_tensor_tensor(
                    scr[:, :w],
                    wb[u % 2][:, :w],
                    0.0,
                    wb[u % 2][:, w : 2 * w],
                    op0=Alu.bypass,
                    op1=Alu.max,
                    accum_out=acc[:, 2 * u : 2 * u + 1],
                ).then_inc(gp_sem, 1)

            # Units near the stream's edges run per-SIDE (pred ops while
            # targ is still in flight — pred lands half a group earlier), so
            # the residual work after the LAST byte arrives is halved. Mid-
            # stream units keep the fused wide ops (fewer op overheads).
            unfused = {0, n_units - 3, n_units - 2, n_units - 1}

            def side_ops(u, s):
                g, o, w = units[u]
                t = inb[g % S_G]
                vector.tensor_max(
                    m1[:, s * w : (s + 1) * w],
                    t[:, s, 0, o : o + w],
                    t[:, s, 1, o : o + w],
                )
                st = vector.scalar_tensor_tensor(
                    ub[u % 2][:, s * w : (s + 1) * w],
                    m1[:, s * w : (s + 1) * w],
                    0.0,
                    t[:, s, 2, o : o + w],
                    op0=Alu.max,
                    op1=Alu.max,
                )
                return st

            for u in range(n_units):
                g, o, w = units[u]
                t = inb[g % S_G]
                if u in unfused:
                    vector.wait_ge(inp_sem[g % 2], 16 * (g // 2 + 1))
                    if u >= 2:
                        # WAR on ub[u%2]: ACT's W of unit u-2 (its reader)
                        vector.wait_ge(act_sem, u - 1)
                    side_ops(u, 0)
                    vector.wait_ge(int_sem[g % 2], 16 * (g // 2 + 1))
                    side_ops(u, 1).then_inc(u_sem, 1)
                    if u > 0:
                        # accum AFTER both sides: its act_sem wait must not
                        # gate the targ-side ops (that serializes the tail)
                        accum(u - 1)
                else:
                    vector.wait_ge(inp_sem[g % 2], 16 * (g // 2 + 1))
                    vector.wait_ge(int_sem[g % 2], 16 * (g // 2 + 1))
                    mv = m1[:, : 2 * w].rearrange("p (s w) -> p s w", s=2)
                    uv = ub[u % 2][:, : 2 * w].rearrange("p (s w) -> p s w", s=2)
                    vector.tensor_max(
                        mv, t[:, :, 0, o : o + w], t[:, :, 1, o : o + w]
                    )
                    if u >= 2:
                        vector.wait_ge(act_sem, u - 1)
                    vector.scalar_tensor_tensor(
                        uv,
                        mv,
                        0.0,
                        t[:, :, 2, o : o + w],
                        op0=Alu.max,
                        op1=Alu.max,
                    ).then_inc(u_sem, 1)
                    accum(u - 1)
            accum(n_units - 1)

        @block.scalar
        def _(scalar):
            # odd units' input DMAs ride the ACT HWDGE ring. Units 1 and 3 go
            # up front (fresh slots, no WAR); unit n+S_G is placed right
            # after ACT(n), whose u_sem wait (>= n+1) covers the WAR for slot
            # (n+S_G) % S_G (last STT reader was unit n).
            for g in range(1, min(S_G, n_groups), 2):
                dma_in(scalar, 0, g)
                dma_in(scalar, 1, g)
            for n in range(n_units):
                w = units[n][2]
                scalar.wait_ge(u_sem, n + 1)
                if n >= 2:
                    # WAR on wb[n%2]: accum of unit n-2 (its reader)
                    scalar.wait_ge(gp_sem, n - 1)
                scalar.activation(
                    wb[n % 2][:, : 2 * w],
                    ub[n % 2][:, : 2 * w],
                    Act.Relu,
                    bias=1.0,
                    scale=-1.0,
                    accum_out=acc[:, 2 * n + 1 : 2 * n + 2],
                ).then_inc(act_sem, 1)
                if n + S_G < n_groups and (n + S_G) % 2 == 1:
                    dma_in(scalar, 0, n + S_G)
                    dma_in(scalar, 1, n + S_G)

        # Skip the Block-exit all-engine barrier (~4.3us): every cross-engine
        # dependency is semaphore-gated and the per-engine exit drains
        # (no_gpsimd_drain path) still fence the DMA rings, so engines may
        # halt independently — NEFF completion waits for all engines anyway.
        nc.all_engine_barrier = lambda *a, **k: None

    del nc.all_engine_barrier  # restore class method
    return nc


_program = None


def _get_program():
    global _program
    if _program is None:
        _program = _build_program()
    return _program


def _finish(partials_list):
    """partials_list: per-core [P, 2*n_units] f32 with cols per unit:
    [sum max(Wp,Wt), sum Wp + sum Wt].
    sum|Vp-Vt| = 2*sum(max) - (sum Wp + sum Wt)."""
    total = np.float64(0.0)
    for p in partials_list:
        p = p.astype(np.float64)
        total += 2.0 * p[:, 0::2].sum() - p[:, 1::2].sum()
    return np.array(total / N_PIX, dtype=np.float32)


def kernel(pred: np.ndarray, target: np.ndarray) -> np.ndarray:
    from concourse.bass_utils import run_bass_kernel_spmd

    nc = _get_program()
    pred = np.ascontiguousarray(pred, dtype=np.float32).reshape(
        N_CORES, N_IMG, C, P, F
    )
    target = np.ascontiguousarray(target, dtype=np.float32).reshape(
        N_CORES, N_IMG, C, P, F
    )
    in_maps = [{"pred": pred[i], "target": target[i]} for i in range(N_CORES)]
    res = run_bass_kernel_spmd(nc, in_maps, list(range(N_CORES)))
    return _finish([r["partials"] for r in res.results])


# revision 25
# speedup vs baseline: 1.0210x; 1.0173x over previous
"""BrightnessLoss Trainium2 kernel (raw Bass, 8-core data parallel).

reference:
    V(x)   = max_c(clip(x, 0, 1))        over channel dim (RGB)
    result = mean(|V(pred) - V(target)|) over (N, H, W)

Identities used on device:
    clip(max(r,g,b),0,1) == max_c(clip(x,0,1))          (clip is monotone)
    W := relu(1 - relu(m)) == 1 - clip(m, 0, 1)
    |Vp - Vt| == |Wp - Wt|
    sum|Wp - Wt| == 2*sum max(Wp,Wt) - sum(Wp + Wt)

The stream is the roofline: ~25.2 MB of fp32 input per core, and the 16
SDMA engines cap at ~24 GB/s each with 4 KB packets (per-packet
overhead), a bit more with 8 KB packets. So the design goal is a gapless
two-ring DMA stream of the largest-possible contiguous runs, with compute
strictly faster than arrival:

  - DMA "groups" cover column ranges of each image; even groups ride the
    Sync HWDGE ring, odd groups the ACT HWDGE ring, each carrying pred
    then targ back-to-back (12.6 MB per ring). Image 0 leads with paired
    small groups (128/128/256/256/640/640 cols) so compute starts ~1 us
    into the stream with both rings' packet sizes matched; image 3
    trails with the mirror (640/640/256/256/128/128) so the closing
    dependency chain is short.
  - 6 group slots [P, 2, 3, w] (both sides side-by-side) keep each ring
    ~3 transfers deep, so the rings never run dry.
  - Compute "units" (<=1024 cols) subdivide groups. Per unit, both sides
    in one wide op:
        DVE TT   m = max(R2, G2)          [P, 2, w]
        DVE STT  u = max(max(m,0), B2)    [P, 2, w]
        ACT      W = Relu(1 - u) (bf16),  accum_out = sum(Wp)+sum(Wt)
        DVE STT  max(Wp, Wt) (bf16),      accum_out = sum
    DVE needs ~5.8 us per 1024-col unit vs ~7.3 us arrival, so it stays
    caught up and the tail after the last packet is just the last small
    unit's chain. Partials go out in two DMAs (bulk early, last units at
    the end). Host combines in float64.
"""

import numpy as np

N_CORES = 8
N_IMG = 4  # 32 / 8
C = 3
P = 128
F = 2048  # 512*512 / 128
N_PIX = 32 * 512 * 512
FC = 1024  # max compute-unit width
S_G = 6  # group slot depth: deep enough that ring issues wait only on
# ancient compute (STT of g-6), keeping both rings' queues full from t=0
# Small transfers are expensive two ways: sub-4KB DRAM runs drop the
# per-SDMA-engine rate, and each transfer carries a ~1.5us ring bubble.
# So the head/tail use the FEWEST small pieces that still let compute
# start early / finish promptly: one 256 + one 768 per edge, staggered
# across the two rings, uniform 1024s everywhere else.
HEAD_SPLIT = (256, 768, 1024)  # image 0 groups (sum = F)
TAIL_SPLIT = (1024, 768, 256)  # last image groups (sum = F)


def _plan():
    """groups: (img, col_off, width); units: (grp_idx, off_in_grp, width).
    One group = one DMA transfer per side = one compute unit (<= FC cols):
    4 KB DRAM runs already saturate the per-SDMA-engine rate, and unit-
    sized transfers keep the slot-WAR release chain fine-grained."""
    groups = []
    for img in range(N_IMG):
        if img == 0:
            widths = HEAD_SPLIT
        elif img == N_IMG - 1:
            widths = TAIL_SPLIT
        else:
            widths = (FC,) * (F // FC)
        o = 0
        for w in widths:
            groups.append((img, o, w))
            o += w
        assert o == F
    units = []
    for g, (_img, _off, w) in enumerate(groups):
        o = 0
        while o < w:
            uw = min(FC, w - o)
            units.append((g, o, uw))
            o += uw
    return groups, units


def _build_program():
    from contextlib import ExitStack

    import concourse.bass as bass
    import concourse.mybir as mybir

    fp32 = mybir.dt.float32
    bf16 = mybir.dt.bfloat16
    Alu = mybir.AluOpType
    Act = mybir.ActivationFunctionType

    groups, units = _plan()
    n_groups = len(groups)
    n_units = len(units)
    last_unit_of = {}
    for u, (g, _o, _w) in enumerate(units):
        last_unit_of[g] = u
    slot_w = [
        max(groups[g][2] for g in range(s, n_groups, S_G)) for s in range(S_G)
    ]

    # detect_race_conditions=False: the raw-mode CoreSim race detector can't
    # see same-engine program-order (DVE m1 -> STT RAW); hardware engines
    # execute in order.
    # The construction-time all_engine_barrier orders the const-tile memsets
    # against engines that read them; this kernel uses only instruction
    # immediates, so skip it and let the engines reach first work sooner.
    _orig_barrier = bass.Bass.all_engine_barrier
    bass.Bass.all_engine_barrier = lambda *a, **k: None
    try:
        nc = bass.Bass(
            "TRN2",
            target_bir_lowering=False,
            debug=False,
            detect_race_conditions=False,
        )
    finally:
        bass.Bass.all_engine_barrier = _orig_barrier
    pred = nc.dram_tensor("pred", [N_IMG, C, P, F], fp32, kind="ExternalInput").ap()
    targ = nc.dram_tensor("target", [N_IMG, C, P, F], fp32, kind="ExternalInput").ap()
    out = nc.dram_tensor(
        "partials", [P, 2 * n_units], fp32, kind="ExternalOutput"
    ).ap()

    with ExitStack() as ctx:
        sb = lambda name, shape, dt=fp32: ctx.enter_context(
            nc.sbuf_tensor(name, shape, dt)
        )
        sem = lambda name: ctx.enter_context(nc.semaphore(name))

        # one slot holds BOTH sides of a group: [P, side, chan, slot_w]
        inb = [sb(f"in{s}", [P, 2, C, slot_w[s]]) for s in range(S_G)]
        ub = [sb(f"u{s}", [P, 2 * FC]) for s in range(2)]
        wb = [sb(f"w{s}", [P, 2 * FC], bf16) for s in range(2)]
        m1 = sb("m1", [P, 3 * FC])  # third segment: hoisted pred-side scratch
        scr = sb("stt_scratch", [P, FC], bf16)
        acc = sb("acc", [P, 2 * n_units])

        inp_sem = [sem("inp0"), sem("inp1")]  # pred side, by ring parity
        int_sem = [sem("int0"), sem("int1")]  # targ side, by ring parity
        u_sem = sem("u")  # +1 per unit after DVE STT (inb consumed)
        act_sem = sem("act")  # +1 per unit after ACT (ub consumed, wb+acc ready)
        gp_sem = sem("gp")  # +1 per unit after DVE accum (wb consumed)
        out_sem = sem("outd")

        def dma_in(eng, side_idx, g):
            img, off, w = groups[g]
            side = (pred, targ)[side_idx]
            s_sem = (inp_sem, int_sem)[side_idx]
            src = side[img, :, :, off : off + w].rearrange("c p f -> p c f")
            eng.dma_start(
                out=inb[g % S_G][:, side_idx, :, :w],
                in_=src,
            ).then_inc(s_sem[g % 2], 16)

        block = ctx.enter_context(nc.Block(no_gpsimd_drain=True))

        @block.sync
        def _(sync):
            # even units ride the SP ring (pred+targ back-to-back); odd units
            # are issued from the ACT stream (second HWDGE ring). The two
            # rings stay one unit apart, which keeps their DRAM read streams
            # decorrelated — issuing each side on its own ring measurably
            # tanks the aggregate rate.
            for g in range(0, n_groups, 2):
                if g >= S_G:
                    # WAR inb[g%S_G]: unit g-S_G's STT was its last reader
                    sync.wait_ge(u_sem, g - S_G + 1)
                dma_in(sync, 0, g)
                dma_in(sync, 1, g)
            if n_units > 2:
                # bulk of partials early; only the last 2 units' cols remain.
                # gp_sem >= k implies act_sem >= k (accum u waits ACT u), so
                # both engines' acc columns for units < k are final.
                sync.wait_ge(gp_sem, n_units - 2)
                sync.dma_start(
                    out=out[:, : 2 * (n_units - 2)],
                    in_=acc[:, : 2 * (n_units - 2)],
                ).then_inc(out_sem, 16)
            sync.wait_ge(gp_sem, n_units)
            # No out_sem wait after the final write: the block-exit drain
            # fences the HWDGE ring before NEFF completion.
            sync.dma_start(
                out=out[:, 2 * max(0, n_units - 2) :],
                in_=acc[:, 2 * max(0, n_units - 2) :],
            ).then_inc(out_sem, 16)

        @block.vector
        def _(vector):
            def accum(u):
                # max(Wp, Wt) elementwise (bf16), accum_out = row sum
                w = units[u][2]
                vector.wait_ge(act_sem, u + 1)
                vector.scalar_tensor_tensor(
                    scr[:, :w],
                    wb[u % 2][:, :w],
                    0.0,
                    wb[u % 2][:, w : 2 * w],
                    op0=Alu.bypass,
                    op1=Alu.max,
                    accum_out=acc[:, 2 * u : 2 * u + 1],
                ).then_inc(gp_sem, 1)

            # Units near the stream's edges run per-SIDE (pred ops while
            # targ is still in flight — pred lands half a group earlier), so
            # the residual work after the LAST byte arrives is halved. Mid-
            # stream units keep the fused wide ops (fewer op overheads).
            unfused = {0, n_units - 3, n_units - 2, n_units - 1}

            def side_ops(u, s, m_off=None):
                g, o, w = units[u]
                t = inb[g % S_G]
                mo = s * w if m_off is None else m_off
                vector.tensor_max(
                    m1[:, mo : mo + w],
                    t[:, s, 0, o : o + w],
                    t[:, s, 1, o : o + w],
                )
                st = vector.scalar_tensor_tensor(
                    ub[u % 2][:, s * w : (s + 1) * w],
                    m1[:, mo : mo + w],
                    0.0,
                    t[:, s, 2, o : o + w],
                    op0=Alu.max,
                    op1=Alu.max,
                )
                return st

            # the pred side of unit n-3 is hoisted before unit n-4: its data
            # lands while DVE waits for n-4's targ half, pulling ~2.4us of
            # work out of the post-stream critical tail. It uses the third
            # m1 segment so n-4's fused scratch isn't clobbered.
            hoist = n_units - 3

            for u in range(n_units):
                g, o, w = units[u]
                t = inb[g % S_G]
                if u == n_units - 4 and hoist in unfused:
                    gh = units[hoist][0]
                    vector.wait_ge(inp_sem[gh % 2], 16 * (gh // 2 + 1))
                    # WAR on ub[hoist%2]: ACT of unit hoist-2 (its reader)
                    vector.wait_ge(act_sem, hoist - 1)
                    side_ops(hoist, 0, m_off=2 * FC)
                if u in unfused:
                    if u == hoist:
                        # pred side already hoisted; targ side only
                        vector.wait_ge(int_sem[g % 2], 16 * (g // 2 + 1))
                        side_ops(u, 1).then_inc(u_sem, 1)
                        accum(u - 1)
                        continue
                    vector.wait_ge(inp_sem[g % 2], 16 * (g // 2 + 1))
                    if u >= 2:
                        # WAR on ub[u%2]: ACT's W of unit u-2 (its reader)
                        vector.wait_ge(act_sem, u - 1)
                    side_ops(u, 0)
                    vector.wait_ge(int_sem[g % 2], 16 * (g // 2 + 1))
                    side_ops(u, 1).then_inc(u_sem, 1)
                    if u > 0:
                        # accum AFTER both sides: its act_sem wait must not
                        # gate the targ-side ops (that serializes the tail)
                        accum(u - 1)
                else:
                    vector.wait_ge(inp_sem[g % 2], 16 * (g // 2 + 1))
                    vector.wait_ge(int_sem[g % 2], 16 * (g // 2 + 1))
                    mv = m1[:, : 2 * w].rearrange("p (s w) -> p s w", s=2)
                    uv = ub[u % 2][:, : 2 * w].rearrange("p (s w) -> p s w", s=2)
                    vector.tensor_max(
                        mv, t[:, :, 0, o : o + w], t[:, :, 1, o : o + w]
                    )
                    if u >= 2:
                        vector.wait_ge(act_sem, u - 1)
                    vector.scalar_tensor_tensor(
                        uv,
                        mv,
                        0.0,
                        t[:, :, 2, o : o + w],
                        op0=Alu.max,
                        op1=Alu.max,
                    ).then_inc(u_sem, 1)
                    accum(u - 1)
            accum(n_units - 1)

        @block.scalar
        def _(scalar):
            # odd units' input DMAs ride the ACT HWDGE ring. Units 1 and 3 go
            # up front (fresh slots, no WAR); unit n+S_G is placed right
            # after ACT(n), whose u_sem wait (>= n+1) covers the WAR for slot
            # (n+S_G) % S_G (last STT reader was unit n).
            for g in range(1, min(S_G, n_groups), 2):
                dma_in(scalar, 0, g)
                dma_in(scalar, 1, g)
            for n in range(n_units):
                w = units[n][2]
                scalar.wait_ge(u_sem, n + 1)
                if n >= 2:
                    # WAR on wb[n%2]: accum of unit n-2 (its reader)
                    scalar.wait_ge(gp_sem, n - 1)
                scalar.activation(
                    wb[n % 2][:, : 2 * w],
                    ub[n % 2][:, : 2 * w],
                    Act.Relu,
                    bias=1.0,
                    scale=-1.0,
                    accum_out=acc[:, 2 * n + 1 : 2 * n + 2],
                ).then_inc(act_sem, 1)
                if n + S_G < n_groups and (n + S_G) % 2 == 1:
                    dma_in(scalar, 0, n + S_G)
                    dma_in(scalar, 1, n + S_G)

        # Skip the Block-exit all-engine barrier (~4.3us): every cross-engine
        # dependency is semaphore-gated and the per-engine exit drains
        # (no_gpsimd_drain path) still fence the DMA rings, so engines may
        # halt independently — NEFF completion waits for all engines anyway.
        nc.all_engine_barrier = lambda *a, **k: None

    del nc.all_engine_barrier  # restore class method
    return nc


_program = None


def _get_program():
    global _program
    if _program is None:
        _program = _build_program()
    return _program


def _finish(partials_list):
    """partials_list: per-core [P, 2*n_units] f32 with cols per unit:
    [sum max(Wp,Wt), sum Wp + sum Wt].
    sum|Vp-Vt| = 2*sum(max) - (sum Wp + sum Wt)."""
    total = np.float64(0.0)
    for p in partials_list:
        p = p.astype(np.float64)
        total += 2.0 * p[:, 0::2].sum() - p[:, 1::2].sum()
    return np.array(total / N_PIX, dtype=np.float32)


def kernel(pred: np.ndarray, target: np.ndarray) -> np.ndarray:
    from concourse.bass_utils import run_bass_kernel_spmd

    nc = _get_program()
    pred = np.ascontiguousarray(pred, dtype=np.float32).reshape(
        N_CORES, N_IMG, C, P, F
    )
    target = np.ascontiguousarray(target, dtype=np.float32).reshape(
        N_CORES, N_IMG, C, P, F
    )
    in_maps = [{"pred": pred[i], "target": target[i]} for i in range(N_CORES)]
    res = run_bass_kernel_spmd(nc, in_maps, list(range(N_CORES)))
    return _finish([r["partials"] for r in res.results])
